# revision 1
# baseline (speedup 1.0000x reference)
"""Multi-headed attention on 8 TRN2 NeuronCores (Bass/Tile).

Problem: x[4, 2048, 1024] f32; 16 heads, Dk=64.
  Q = x@Wq+bq, K = x@Wk+bk, V = x@Wv+bv  (per-head split)
  out = softmax(QK^T/8) V  re-merged, @Wo + bo

Sharding (tensor-parallel heads x batch): core = b*2 + hg
  b  in 0..3  : batch index
  hg in 0..1  : head group (8 heads = 512 of the 1024 d_model dims)
Each core gets x[b]^T (pre-transposed on host, bf16) and the hg-slice of the
weights, and produces the partial Y^T = (P V_hg) @ Wo_hg  (d-major, f32,
no biases). Host sums the two head-group partials per batch, transposes, and
adds bo + bv@Wo (the V-bias commutes through softmax: rows of P sum to 1).

On-core dataflow (all matmul operands bf16, PSUM f32):
  Xt   [1024,2048] d-major input (host-provided)
  Qt,Kt[512,2048]  d-major projections; bias added during PSUM->SBUF copy
  Vaug [2048, 8,65] natural V with a ones column per head (rowsum trick)
  per (q-block 512, head-pair): St^T [128k,2x512q] psum  (2 heads packed in
     the 128-row PE array via base-partition 0/64 row tiling, K=64 each)
  P~ = exp(St^T / 8) -> bf16 (one ACT op per [128,1024] tile; no max-sub:
     |scores| <~ 2 for this problem's distribution, exp is safe in f32)
  O^T+rowsum = [V_h | 1]^T @ P~^T  accumulated over 16 k-chunks -> [65, 512]
  Ot = O^T * (1/rowsum broadcast)  -> bf16  (odd heads DMA-shifted to
     partitions 64..127 so the final matmul sees full 128-row d-chunks)
  Y^T = Wo^T @ Ot  accumulated over 4 d-chunks -> f32 -> DRAM
"""

import os
import numpy as np
import ml_dtypes
from contextlib import ExitStack

import jax
from jax.sharding import Mesh, PartitionSpec
from jax.experimental.shard_map import shard_map

import concourse.bass as bass
import concourse.tile as tile
from concourse import bacc, mybir
from concourse import bass2jax

BF16 = ml_dtypes.bfloat16

B, S, D, H, DK = 4, 2048, 1024, 16, 64
HPG = 8              # heads per group (per core)
DS = HPG * DK        # 512: d_model slice per core
N_CORES = 8
P = 128
QW = 512             # q block width
QB = S // QW         # 4 q blocks
KC = D // P          # 8 contraction chunks for projections
DC = DS // P         # 4 d-chunks of the head-group slice (= head pairs)
TC = S // P          # 16 token chunks (= k_tok chunks)
FP32 = mybir.dt.float32
BF = mybir.dt.bfloat16
AF = mybir.ActivationFunctionType


# ablation switches for performance bisection (all True = full kernel)
ABLATE = {"exp": True, "pv": True, "norm": True, "final": True}


def build_tile_kernel(ctx: ExitStack, tc_ctx: tile.TileContext,
                      xt, wq, wk, wv, wo, bq, bk, yt, repeat=1):
    nc = tc_ctx.nc
    tc = tc_ctx

    wpool = ctx.enter_context(tc.tile_pool(name="w", bufs=1))
    xpool = ctx.enter_context(tc.tile_pool(name="x", bufs=1))
    qkpool = ctx.enter_context(tc.tile_pool(name="qk", bufs=1))
    vpool = ctx.enter_context(tc.tile_pool(name="v", bufs=1))
    opool = ctx.enter_context(tc.tile_pool(name="o", bufs=1))
    ptpool = ctx.enter_context(tc.tile_pool(name="pt", bufs=22))
    small = ctx.enter_context(tc.tile_pool(name="small", bufs=3))
    ypool = ctx.enter_context(tc.tile_pool(name="y", bufs=3))
    psA = ctx.enter_context(tc.tile_pool(name="psA", bufs=2, space="PSUM"))
    psB = ctx.enter_context(tc.tile_pool(name="psB", bufs=4, space="PSUM"))
    dscr = ctx.enter_context(tc.tile_pool(name="dscr", bufs=4, space="DRAM"))

    # ---- inputs -> SBUF (x and Wq/Wk first: they gate the first
    # projections and therefore the start of the exp stream) ----
    w_q = wpool.tile([P, KC, DS], BF)
    w_k = wpool.tile([P, KC, DS], BF)
    w_v = wpool.tile([P, KC, DS], BF)
    x_sb = xpool.tile([P, KC, S], BF)
    for kc in range(KC):
        nc.sync.dma_start(x_sb[:, kc], xt[kc * P:(kc + 1) * P, :])
        nc.sync.dma_start(w_q[:, kc], wq[kc * P:(kc + 1) * P, :])
        nc.sync.dma_start(w_k[:, kc], wk[kc * P:(kc + 1) * P, :])
    for kc in range(KC):
        nc.sync.dma_start(w_v[:, kc], wv[kc * P:(kc + 1) * P, :])
    w_o = wpool.tile([P, DC, D], BF)
    for dc in range(DC):
        nc.sync.dma_start(w_o[:, dc], wo[dc * P:(dc + 1) * P, :])
    bq_sb = wpool.tile([P, DC], FP32)
    bk_sb = wpool.tile([P, DC], FP32)
    nc.sync.dma_start(bq_sb[:], bq.rearrange("(c p) -> p c", p=P))
    nc.sync.dma_start(bk_sb[:], bk.rearrange("(c p) -> p c", p=P))

    qt = qkpool.tile([P, DC, S], BF)
    kt = qkpool.tile([P, DC, S], BF)
    vaug = vpool.tile([P, TC, HPG, DK + 1], BF)
    ot = opool.tile([P, DC, S], BF)

    nc.vector.memset(vaug[:, :, :, DK], 1.0)  # ones column for rowsums

    def qk_proj_chunk(c):
        """Project d_out chunk c of Q^T and K^T (128 dims x all 2048 tokens)."""
        for w_sb, b_sb, dest in ((w_q, bq_sb, qt), (w_k, bk_sb, kt)):
            for tb in range(QB):
                ps = psB.tile([P, QW], FP32, tag="b")
                for kc in range(KC):
                    nc.tensor.matmul(
                        ps[:],
                        lhsT=w_sb[:, kc, c * P:(c + 1) * P],
                        rhs=x_sb[:, kc, tb * QW:(tb + 1) * QW],
                        start=(kc == 0), stop=(kc == KC - 1))
                nc.vector.tensor_scalar_add(
                    dest[:, c, tb * QW:(tb + 1) * QW], ps[:], b_sb[:, c:c + 1])

    def v_proj():
        for tci in range(TC):
            ps = psB.tile([P, DS], FP32, tag="b")
            for kc in range(KC):
                nc.tensor.matmul(
                    ps[:],
                    lhsT=x_sb[:, kc, tci * P:(tci + 1) * P],
                    rhs=w_v[:, kc, :],
                    start=(kc == 0), stop=(kc == KC - 1))
            nc.vector.tensor_copy(
                vaug[:, tci, :, 0:DK], ps.rearrange("p (h e) -> p h e", e=DK))

    def attention_unit(qb, j):
        """scores + exp + PV + normalize for head pair j, q block qb."""
        pts = []
        for kc2 in range(TC):
            psS = psA.tile([P, 2 * QW], FP32, tag="s")
            for h01 in range(2):
                lo = h01 * DK
                nc.tensor.matmul(
                    psS[:, h01 * QW:(h01 + 1) * QW],
                    lhsT=kt[lo:lo + DK, j, kc2 * P:(kc2 + 1) * P],
                    rhs=qt[lo:lo + DK, j, qb * QW:(qb + 1) * QW],
                    start=True, stop=True)
            if ABLATE["exp"]:
                pt = ptpool.tile([P, 2 * QW], BF, tag="pt")
                nc.scalar.activation(pt[:], psS[:], AF.Exp, scale=0.125)
                pts.append(pt)
        if not (ABLATE["exp"] and ABLATE["pv"]):
            return
        # both heads' PV chains interleaved per k-chunk: each P~ tile is
        # fully consumed at its own k-step (released for the next unit's
        # exps immediately) and the two PSUM accumulations overlap on PE
        psOs = [psB.tile([P, QW], FP32, tag="b", name=f"psO{_h}")
                for _h in range(2)]
        for kc2 in range(TC):
            for h01 in range(2):
                nc.tensor.matmul(
                    psOs[h01][0:DK + 1, :],
                    lhsT=vaug[:, kc2, 2 * j + h01, :],
                    rhs=pts[kc2][:, h01 * QW:(h01 + 1) * QW],
                    start=(kc2 == 0), stop=(kc2 == TC - 1))
        for h01 in range(2):
            h = 2 * j + h01
            psO = psOs[h01]
            if not ABLATE["norm"]:
                # timing-ablation path: skip normalization, copy raw O
                # (partition-preserving; wrong results, right timing shape)
                nc.vector.tensor_copy(
                    ot[0:DK, j, qb * QW:(qb + 1) * QW], psO[0:DK, :])
                continue
            # one quick copy of O+rowsum to SBUF releases the PSUM slot
            # (~0.7us) instead of holding it through the multi-hop DMA
            # normalization chain (~5us)
            ou = small.tile([P, QW], FP32, tag="ou")
            nc.vector.tensor_copy(ou[0:DK + 1, :], psO[0:DK + 1, :])
            # 1/rowsum, computed on 64 partitions instead of one: DVE
            # reciprocal is ~6ns/elem *per lane*, so a [1,512] row costs
            # 3.1us while [64,8] costs ~0.2us. Rowsum row -> DRAM ->
            # reload as [64,8] -> reciprocal -> DRAM -> broadcast [64,512].
            # (SBUF APs cannot repartition or broadcast; DRAM hops can.
            # reciprocal_approx_fast / gpsimd.partition_broadcast are
            # broken on this runtime — standard ops only.)
            rd = dscr.tile([1, QW], FP32, tag="rd")
            nc.sync.dma_start(rd[:], ou[DK:DK + 1, :])
            rec = small.tile([P, 8], FP32, tag="rec")
            nc.sync.dma_start(rec[0:DK, :],
                              rd.rearrange("o (a b) -> (o a) b", b=8))
            nc.vector.reciprocal(rec[0:DK, :], rec[0:DK, :])
            rd2 = dscr.tile([1, QW], FP32, tag="rd2")
            nc.sync.dma_start(rd2.rearrange("o (a b) -> (o a) b", b=8),
                              rec[0:DK, :])
            bc = small.tile([P, QW], FP32, tag="bc")
            nc.sync.dma_start(bc[0:DK, :], rd2.to_broadcast((DK, QW)))
            if h01 == 0:
                nc.vector.tensor_mul(
                    ot[0:DK, j, qb * QW:(qb + 1) * QW],
                    ou[0:DK, :], bc[0:DK, :])
            else:
                tmp = small.tile([P, QW], BF, tag="tmp")
                nc.vector.tensor_mul(tmp[0:DK, :], ou[0:DK, :], bc[0:DK, :])
                nc.sync.dma_start(
                    ot[DK:P, j, qb * QW:(qb + 1) * QW], tmp[0:DK, :])

    def final_qb(qb):
        for oc in range(D // P):
            ps = psB.tile([P, QW], FP32, tag="b")
            for dc in range(DC):
                nc.tensor.matmul(
                    ps[:],
                    lhsT=w_o[:, dc, oc * P:(oc + 1) * P],
                    rhs=ot[:, dc, qb * QW:(qb + 1) * QW],
                    start=(dc == 0), stop=(dc == DC - 1))
            y_sb = ypool.tile([P, QW], FP32, tag="y")
            # explicit DVE: finals now run alongside exps, and nc.any would
            # put these copies on the exp-critical ACT engine
            nc.vector.tensor_copy(y_sb[:], ps[:])
            nc.sync.dma_start(
                yt[oc * P:(oc + 1) * P, qb * QW:(qb + 1) * QW], y_sb[:])

    def compute_once():
        # Emission order == scheduler priority: get head-pair 0's Q/K and V
        # done first so the ACT exp stream (the bottleneck engine) starts as
        # early as possible; later head-pairs' projections are interleaved
        # between attention units.
        qk_proj_chunk(0)
        v_proj()
        # qb-major: q-block 0 finishes all head pairs after 4 units, so the
        # output projections and their 8MB DMA spread across the kernel
        # instead of bunching at the tail; remaining Q/K projections hide
        # under q-block 0's exps.
        for qb in range(QB):
            for j in range(DC):
                if qb == 0 and j > 0:
                    qk_proj_chunk(j)
                attention_unit(qb, j)
            if ABLATE["final"]:
                final_qb(qb)

    for _ in range(repeat):
        compute_once()


def build_module(repeat=1):
    nc = bacc.Bacc("TRN2", target_bir_lowering=False, debug=False)
    xt = nc.dram_tensor("xt", [D, S], BF, kind="ExternalInput").ap()
    wq = nc.dram_tensor("wq", [D, DS], BF, kind="ExternalInput").ap()
    wk = nc.dram_tensor("wk", [D, DS], BF, kind="ExternalInput").ap()
    wv = nc.dram_tensor("wv", [D, DS], BF, kind="ExternalInput").ap()
    wo = nc.dram_tensor("wo", [DS, D], BF, kind="ExternalInput").ap()
    bq = nc.dram_tensor("bq", [DS], FP32, kind="ExternalInput").ap()
    bk = nc.dram_tensor("bk", [DS], FP32, kind="ExternalInput").ap()
    yt = nc.dram_tensor("yt", [D, S], FP32, kind="ExternalOutput").ap()
    with tile.TileContext(nc) as tc:
        with ExitStack() as ctx:
            build_tile_kernel(ctx, tc, xt, wq, wk, wv, wo, bq, bk, yt,
                              repeat=repeat)
    nc.compile()
    return nc


def _collect_io(nc):
    partition_name = (nc.partition_id_tensor.name
                      if nc.partition_id_tensor else None)
    in_names, out_names, out_avals = [], [], []
    for alloc in nc.m.functions[0].allocations:
        if not isinstance(alloc, mybir.MemoryLocationSet):
            continue
        name = alloc.memorylocations[0].name
        if alloc.kind == "ExternalInput":
            if name != partition_name:
                in_names.append(name)
        elif alloc.kind == "ExternalOutput":
            out_names.append(name)
            out_avals.append(jax.core.ShapedArray(
                tuple(alloc.tensor_shape), mybir.dt.np(alloc.dtype)))
    return in_names, out_names, out_avals, partition_name


def make_runner(nc, donate=False):
    """Multi-core PJRT runner (the run_bass_via_pjrt path, but with the
    jitted executable retained so repeated calls don't re-lower).

    donate=False: the kernel writes every element of its outputs, so the
    zero output-operands never need to be donated; keeping them allows the
    same device-resident args to be re-used for repeated timed calls."""
    bass2jax.install_neuronx_cc_hook()
    in_names, out_names, out_avals, partition_name = _collect_io(nc)
    n_params, n_outs = len(in_names), len(out_names)
    all_names = in_names + out_names
    if partition_name is not None:
        all_names = all_names + [partition_name]

    def _body(*args):
        operands = list(args)
        if partition_name is not None:
            operands.append(bass2jax.partition_id_tensor())
        outs = bass2jax._bass_exec_p.bind(
            *operands,
            out_avals=tuple(out_avals),
            in_names=tuple(all_names),
            out_names=tuple(out_names),
            lowering_input_output_aliases=(),
            sim_require_finite=True,
            sim_require_nnan=True,
            nc=nc,
        )
        return tuple(outs)

    devices = jax.devices()[:N_CORES]
    mesh = Mesh(np.asarray(devices), ("core",))
    jit_kwargs = dict(keep_unused=True)
    if donate:
        jit_kwargs["donate_argnums"] = tuple(range(n_params, n_params + n_outs))
    sharded = jax.jit(
        shard_map(_body, mesh=mesh,
                  in_specs=(PartitionSpec("core"),) * (n_params + n_outs),
                  out_specs=(PartitionSpec("core"),) * n_outs,
                  check_rep=False),
        **jit_kwargs)

    def host_args(in_maps):
        concat_in = [
            np.concatenate([np.asarray(m[name]) for m in in_maps], axis=0)
            for name in in_names]
        concat_zeros = [
            np.zeros((N_CORES * a.shape[0],) + tuple(a.shape[1:]), a.dtype)
            for a in out_avals]
        return concat_in + concat_zeros

    def device_args(in_maps):
        from jax.sharding import NamedSharding
        args = host_args(in_maps)
        return [
            jax.device_put(a, NamedSharding(
                mesh, PartitionSpec("core", *(None,) * (a.ndim - 1))))
            for a in args]

    def run(in_maps, args=None):
        if args is None:
            args = host_args(in_maps)
        out_arrs = sharded(*args)
        return [
            {name: np.asarray(out_arrs[i]).reshape(
                (N_CORES,) + tuple(out_avals[i].shape))[c]
             for i, name in enumerate(out_names)}
            for c in range(N_CORES)]

    run.in_names = in_names
    run.out_names = out_names
    run.out_avals = out_avals
    run.sharded = sharded
    run.mesh = mesh
    run.host_args = host_args
    run.device_args = device_args
    return run


def shard_inputs(inputs):
    """Full problem inputs -> 8 per-core input maps (host-side prep)."""
    x = np.asarray(inputs["x"], dtype=np.float32)
    Wq = np.asarray(inputs["Wq"], dtype=np.float32)
    Wk = np.asarray(inputs["Wk"], dtype=np.float32)
    Wv = np.asarray(inputs["Wv"], dtype=np.float32)
    Wo = np.asarray(inputs["Wo"], dtype=np.float32)
    bq = np.asarray(inputs["bq"], dtype=np.float32)
    bk = np.asarray(inputs["bk"], dtype=np.float32)
    in_maps = []
    for b in range(B):
        xt_b = np.ascontiguousarray(x[b].T).astype(BF16)
        for hg in range(2):
            sl = slice(hg * DS, (hg + 1) * DS)
            in_maps.append({
                "xt": xt_b,
                "wq": np.ascontiguousarray(Wq[:, sl]).astype(BF16),
                "wk": np.ascontiguousarray(Wk[:, sl]).astype(BF16),
                "wv": np.ascontiguousarray(Wv[:, sl]).astype(BF16),
                "wo": np.ascontiguousarray(Wo[sl, :]).astype(BF16),
                "bq": np.ascontiguousarray(bq[sl]),
                "bk": np.ascontiguousarray(bk[sl]),
            })
    return in_maps


def gather_output(results, inputs):
    Wo = np.asarray(inputs["Wo"], dtype=np.float32)
    bv = np.asarray(inputs["bv"], dtype=np.float32)
    bo = np.asarray(inputs["bo"], dtype=np.float32)
    bias = bo + bv @ Wo  # V-bias passes through softmax (rows of P sum to 1)
    out = np.empty((B, S, D), dtype=np.float32)
    for b in range(B):
        acc = results[2 * b]["yt"] + results[2 * b + 1]["yt"]  # [D, S]
        out[b] = acc.T + bias
    return out


_CACHE = {}


def _get_runner():
    if "runner" not in _CACHE:
        nc = build_module()
        _CACHE["nc"] = nc
        _CACHE["runner"] = make_runner(nc)
    return _CACHE["runner"]


def kernel(**inputs) -> np.ndarray:
    runner = _get_runner()
    in_maps = shard_inputs(inputs)
    results = runner(in_maps)
    return gather_output(results, inputs)



# revision 11
# speedup vs baseline: 1.1327x; 1.1327x over previous
"""Multi-headed attention on 8 TRN2 NeuronCores (Bass/Tile).

Problem: x[4, 2048, 1024] f32; 16 heads, Dk=64.
  Q = x@Wq+bq, K = x@Wk+bk, V = x@Wv+bv  (per-head split)
  out = softmax(QK^T/8) V  re-merged, @Wo + bo

Sharding (tensor-parallel heads x batch): core = b*2 + hg
  b  in 0..3  : batch index
  hg in 0..1  : head group (8 heads = 512 of the 1024 d_model dims)
Each core gets x[b]^T (pre-transposed on host, bf16) and the hg-slice of the
weights, and produces the partial Y^T = (P V_hg) @ Wo_hg  (d-major, f32,
no biases). Host sums the two head-group partials per batch, transposes, and
adds bo + bv@Wo (the V-bias commutes through softmax: rows of P sum to 1).

On-core dataflow (all matmul operands bf16, PSUM f32):
  Xt   [1024,2048] d-major input (host-provided)
  Qt,Kt[512,2048]  d-major projections; bias added during PSUM->SBUF copy
  Vaug [2048, 8,65] natural V with a ones column per head (rowsum trick)
  per (q-block 512, head-pair): St^T [128k,2x512q] psum  (2 heads packed in
     the 128-row PE array via base-partition 0/64 row tiling, K=64 each)
  P~ = exp(St^T / 8) -> bf16 (one ACT op per [128,1024] tile; no max-sub:
     |scores| <~ 2 for this problem's distribution, exp is safe in f32)
  O^T+rowsum = [V_h | 1]^T @ P~^T  accumulated over 16 k-chunks -> [65, 512]
  Ot = O^T * (1/rowsum broadcast)  -> bf16  (odd heads DMA-shifted to
     partitions 64..127 so the final matmul sees full 128-row d-chunks)
  Y^T = Wo^T @ Ot  accumulated over 4 d-chunks -> f32 -> DRAM
"""

import os
import numpy as np
import ml_dtypes
from contextlib import ExitStack

import jax
from jax.sharding import Mesh, PartitionSpec
from jax.experimental.shard_map import shard_map

import concourse.bass as bass
import concourse.tile as tile
from concourse import bacc, mybir
from concourse import bass2jax

BF16 = ml_dtypes.bfloat16

B, S, D, H, DK = 4, 2048, 1024, 16, 64
HPG = 8              # heads per group (per core)
DS = HPG * DK        # 512: d_model slice per core
N_CORES = 8
P = 128
QW = 512             # q block width
QB = S // QW         # 4 q blocks
KC = D // P          # 8 contraction chunks for projections
DC = DS // P         # 4 d-chunks of the head-group slice (= head pairs)
TC = S // P          # 16 token chunks (= k_tok chunks)
FP32 = mybir.dt.float32
BF = mybir.dt.bfloat16
F8 = mybir.dt.float8e4
AF = mybir.ActivationFunctionType


# ablation switches for performance bisection (all True = full kernel)
ABLATE = {"exp": True, "pv": True, "norm": True, "final": True}


def build_tile_kernel(ctx: ExitStack, tc_ctx: tile.TileContext,
                      xt, wq, wk, wv, wo, bq, bk, yt, repeat=1):
    nc = tc_ctx.nc
    tc = tc_ctx

    wpool = ctx.enter_context(tc.tile_pool(name="w", bufs=1))
    xpool = ctx.enter_context(tc.tile_pool(name="x", bufs=1))
    qkpool = ctx.enter_context(tc.tile_pool(name="qk", bufs=1))
    vpool = ctx.enter_context(tc.tile_pool(name="v", bufs=1))
    opool = ctx.enter_context(tc.tile_pool(name="o", bufs=1))
    ptpool = ctx.enter_context(tc.tile_pool(name="pt", bufs=22))
    small = ctx.enter_context(tc.tile_pool(name="small", bufs=3))
    ypool = ctx.enter_context(tc.tile_pool(name="y", bufs=3))
    psA = ctx.enter_context(tc.tile_pool(name="psA", bufs=2, space="PSUM"))
    psB = ctx.enter_context(tc.tile_pool(name="psB", bufs=4, space="PSUM"))
    dscr = ctx.enter_context(tc.tile_pool(name="dscr", bufs=4, space="DRAM"))

    # ---- inputs -> SBUF (x and Wq/Wk first: they gate the first
    # projections and therefore the start of the exp stream) ----
    w_q = wpool.tile([P, KC, DS], BF)
    w_k = wpool.tile([P, KC, DS], BF)
    w_v = wpool.tile([P, KC, DS], BF)
    x_sb = xpool.tile([P, KC, S], BF)
    for kc in range(KC):
        nc.sync.dma_start(x_sb[:, kc], xt[kc * P:(kc + 1) * P, :])
        nc.sync.dma_start(w_q[:, kc], wq[kc * P:(kc + 1) * P, :])
        nc.sync.dma_start(w_k[:, kc], wk[kc * P:(kc + 1) * P, :])
    for kc in range(KC):
        nc.sync.dma_start(w_v[:, kc], wv[kc * P:(kc + 1) * P, :])
    w_o = wpool.tile([P, DC, D], BF)
    for dc in range(DC):
        nc.sync.dma_start(w_o[:, dc], wo[dc * P:(dc + 1) * P, :])
    bq_sb = wpool.tile([P, DC], FP32)
    bk_sb = wpool.tile([P, DC], FP32)
    nc.sync.dma_start(bq_sb[:], bq.rearrange("(c p) -> p c", p=P))
    nc.sync.dma_start(bk_sb[:], bk.rearrange("(c p) -> p c", p=P))

    qt = qkpool.tile([P, DC, S], BF)
    kt = qkpool.tile([P, DC, S], BF)
    # V for fp8 DoubleRow PV: [double-chunk, head, k-plane, 128 cols] where
    # cols 0:64 = V dims, col 64 = ones (rowsum trick), cols 65:128 = zeros
    # (dual-fp8 ldweights requires 128 stationary columns; zero-pad so the
    # junk psum rows 65:127 stay finite).  k-token = 256*c + 128*plane + p.
    # Each (c, h) slice is a contiguous [2, 128] block: the dual-fp8
    # ldweights path needs plane-stride == column count.
    vaug = vpool.tile([P, TC // 2, HPG, 2, P], F8)
    ot = opool.tile([P, DC, S], BF)

    nc.vector.memset(vaug[:, :, :, :, DK:], 0.0)
    nc.vector.memset(vaug[:, :, :, :, DK], 1.0)  # ones column for rowsums

    def qk_proj_chunk(c):
        """Project d_out chunk c of Q^T and K^T (128 dims x all 2048 tokens)."""
        for w_sb, b_sb, dest in ((w_q, bq_sb, qt), (w_k, bk_sb, kt)):
            for tb in range(QB):
                ps = psB.tile([P, QW], FP32, tag="b")
                for kc in range(KC):
                    nc.tensor.matmul(
                        ps[:],
                        lhsT=w_sb[:, kc, c * P:(c + 1) * P],
                        rhs=x_sb[:, kc, tb * QW:(tb + 1) * QW],
                        start=(kc == 0), stop=(kc == KC - 1))
                nc.vector.tensor_scalar_add(
                    dest[:, c, tb * QW:(tb + 1) * QW], ps[:], b_sb[:, c:c + 1])

    def v_proj():
        for tci in range(TC):
            ps = psB.tile([P, DS], FP32, tag="b")
            for kc in range(KC):
                nc.tensor.matmul(
                    ps[:],
                    lhsT=x_sb[:, kc, tci * P:(tci + 1) * P],
                    rhs=w_v[:, kc, :],
                    start=(kc == 0), stop=(kc == KC - 1))
            nc.vector.tensor_copy(
                vaug[:, tci // 2, :, tci % 2, 0:DK],
                ps.rearrange("p (h e) -> p h e", e=DK))

    def attention_unit(qb, j):
        """scores + exp + PV + normalize for head pair j, q block qb."""
        pts = []
        for kc2 in range(TC):
            psS = psA.tile([P, 2 * QW], FP32, tag="s")
            for h01 in range(2):
                lo = h01 * DK
                nc.tensor.matmul(
                    psS[:, h01 * QW:(h01 + 1) * QW],
                    lhsT=kt[lo:lo + DK, j, kc2 * P:(kc2 + 1) * P],
                    rhs=qt[lo:lo + DK, j, qb * QW:(qb + 1) * QW],
                    start=True, stop=True)
            if ABLATE["exp"]:
                # fp8 P~, two 128-token k-planes per tile (DoubleRow layout);
                # [head, plane, q] so each head's [2, 512] planes-pair is
                # contiguous for the PV moving operand
                if kc2 % 2 == 0:
                    pt = ptpool.tile([P, 2, 2, QW], F8, tag="pt")
                    pts.append(pt)
                nc.scalar.activation(pts[-1][:, :, kc2 % 2, :], psS[:],
                                     AF.Exp, scale=0.125)
        if not (ABLATE["exp"] and ABLATE["pv"]):
            return
        # both heads' PV chains interleaved per double-k-chunk: fp8 DoubleRow
        # matmuls (2x bf16 rate); each P~ tile is fully consumed at its own
        # k-step and the accumulations overlap on PE
        psOs = [psB.tile([P, QW], FP32, tag="b", name=f"psO{_h}")
                for _h in range(2)]
        # one accumulation group per head tile: start/stop only at the global
        # first/last matmul (psum "zero regions" are 2KB = the whole tile; a
        # second start would lazily re-zero the sibling slab's partial sums)
        for c in range(TC // 2):
            for h01 in range(2):
                for sl in range(2):
                    nc.tensor.matmul(
                        psOs[h01][:, sl * 256:(sl + 1) * 256],
                        lhsT=vaug[:, c, 2 * j + h01, :, :],
                        rhs=pts[c][:, h01, :, sl * 256:(sl + 1) * 256],
                        perf_mode=mybir.MatmulPerfMode.DoubleRow,
                        start=(c == 0 and sl == 0),
                        stop=(c == TC // 2 - 1 and sl == 1))
        for h01 in range(2):
            h = 2 * j + h01
            psO = psOs[h01]
            if not ABLATE["norm"]:
                # timing-ablation path: skip normalization, copy raw O
                # (partition-preserving; wrong results, right timing shape)
                nc.vector.tensor_copy(
                    ot[0:DK, j, qb * QW:(qb + 1) * QW], psO[0:DK, :])
                continue
            # one quick copy of O+rowsum to SBUF releases the PSUM slot
            # (~0.7us) instead of holding it through the multi-hop DMA
            # normalization chain (~5us)
            ou = small.tile([P, QW], FP32, tag="ou")
            nc.vector.tensor_copy(ou[0:DK + 1, :], psO[0:DK + 1, :])
            # 1/rowsum, computed on 64 partitions instead of one: DVE
            # reciprocal is ~6ns/elem *per lane*, so a [1,512] row costs
            # 3.1us while [64,8] costs ~0.2us. Rowsum row -> DRAM ->
            # reload as [64,8] -> reciprocal -> DRAM -> broadcast [64,512].
            # (SBUF APs cannot repartition or broadcast; DRAM hops can.
            # reciprocal_approx_fast / gpsimd.partition_broadcast are
            # broken on this runtime — standard ops only.)
            rd = dscr.tile([1, QW], FP32, tag="rd")
            nc.sync.dma_start(rd[:], ou[DK:DK + 1, :])
            rec = small.tile([P, 8], FP32, tag="rec")
            nc.sync.dma_start(rec[0:DK, :],
                              rd.rearrange("o (a b) -> (o a) b", b=8))
            nc.vector.reciprocal(rec[0:DK, :], rec[0:DK, :])
            rd2 = dscr.tile([1, QW], FP32, tag="rd2")
            nc.sync.dma_start(rd2.rearrange("o (a b) -> (o a) b", b=8),
                              rec[0:DK, :])
            bc = small.tile([P, QW], FP32, tag="bc")
            nc.sync.dma_start(bc[0:DK, :], rd2.to_broadcast((DK, QW)))
            if h01 == 0:
                nc.vector.tensor_mul(
                    ot[0:DK, j, qb * QW:(qb + 1) * QW],
                    ou[0:DK, :], bc[0:DK, :])
            else:
                tmp = small.tile([P, QW], BF, tag="tmp")
                nc.vector.tensor_mul(tmp[0:DK, :], ou[0:DK, :], bc[0:DK, :])
                nc.sync.dma_start(
                    ot[DK:P, j, qb * QW:(qb + 1) * QW], tmp[0:DK, :])

    def final_qb(qb):
        for oc in range(D // P):
            ps = psB.tile([P, QW], FP32, tag="b")
            for dc in range(DC):
                nc.tensor.matmul(
                    ps[:],
                    lhsT=w_o[:, dc, oc * P:(oc + 1) * P],
                    rhs=ot[:, dc, qb * QW:(qb + 1) * QW],
                    start=(dc == 0), stop=(dc == DC - 1))
            y_sb = ypool.tile([P, QW], FP32, tag="y")
            # explicit DVE: finals now run alongside exps, and nc.any would
            # put these copies on the exp-critical ACT engine
            nc.vector.tensor_copy(y_sb[:], ps[:])
            nc.sync.dma_start(
                yt[oc * P:(oc + 1) * P, qb * QW:(qb + 1) * QW], y_sb[:])

    def compute_once():
        # Emission order == scheduler priority: get head-pair 0's Q/K and V
        # done first so the ACT exp stream (the bottleneck engine) starts as
        # early as possible; later head-pairs' projections are interleaved
        # between attention units.
        qk_proj_chunk(0)
        v_proj()
        # qb-major: q-block 0 finishes all head pairs after 4 units, so the
        # output projections and their 8MB DMA spread across the kernel
        # instead of bunching at the tail; remaining Q/K projections hide
        # under q-block 0's exps.
        for qb in range(QB):
            for j in range(DC):
                if qb == 0 and j > 0:
                    qk_proj_chunk(j)
                attention_unit(qb, j)
            if ABLATE["final"]:
                final_qb(qb)

    for _ in range(repeat):
        compute_once()


def build_module(repeat=1):
    nc = bacc.Bacc("TRN2", target_bir_lowering=False, debug=False)
    xt = nc.dram_tensor("xt", [D, S], BF, kind="ExternalInput").ap()
    wq = nc.dram_tensor("wq", [D, DS], BF, kind="ExternalInput").ap()
    wk = nc.dram_tensor("wk", [D, DS], BF, kind="ExternalInput").ap()
    wv = nc.dram_tensor("wv", [D, DS], BF, kind="ExternalInput").ap()
    wo = nc.dram_tensor("wo", [DS, D], BF, kind="ExternalInput").ap()
    bq = nc.dram_tensor("bq", [DS], FP32, kind="ExternalInput").ap()
    bk = nc.dram_tensor("bk", [DS], FP32, kind="ExternalInput").ap()
    yt = nc.dram_tensor("yt", [D, S], FP32, kind="ExternalOutput").ap()
    with tile.TileContext(nc) as tc:
        with ExitStack() as ctx:
            build_tile_kernel(ctx, tc, xt, wq, wk, wv, wo, bq, bk, yt,
                              repeat=repeat)
    nc.compile()
    return nc


def _collect_io(nc):
    partition_name = (nc.partition_id_tensor.name
                      if nc.partition_id_tensor else None)
    in_names, out_names, out_avals = [], [], []
    for alloc in nc.m.functions[0].allocations:
        if not isinstance(alloc, mybir.MemoryLocationSet):
            continue
        name = alloc.memorylocations[0].name
        if alloc.kind == "ExternalInput":
            if name != partition_name:
                in_names.append(name)
        elif alloc.kind == "ExternalOutput":
            out_names.append(name)
            out_avals.append(jax.core.ShapedArray(
                tuple(alloc.tensor_shape), mybir.dt.np(alloc.dtype)))
    return in_names, out_names, out_avals, partition_name


def make_runner(nc, donate=False):
    """Multi-core PJRT runner (the run_bass_via_pjrt path, but with the
    jitted executable retained so repeated calls don't re-lower).

    donate=False: the kernel writes every element of its outputs, so the
    zero output-operands never need to be donated; keeping them allows the
    same device-resident args to be re-used for repeated timed calls."""
    bass2jax.install_neuronx_cc_hook()
    in_names, out_names, out_avals, partition_name = _collect_io(nc)
    n_params, n_outs = len(in_names), len(out_names)
    all_names = in_names + out_names
    if partition_name is not None:
        all_names = all_names + [partition_name]

    def _body(*args):
        operands = list(args)
        if partition_name is not None:
            operands.append(bass2jax.partition_id_tensor())
        outs = bass2jax._bass_exec_p.bind(
            *operands,
            out_avals=tuple(out_avals),
            in_names=tuple(all_names),
            out_names=tuple(out_names),
            lowering_input_output_aliases=(),
            sim_require_finite=True,
            sim_require_nnan=True,
            nc=nc,
        )
        return tuple(outs)

    devices = jax.devices()[:N_CORES]
    mesh = Mesh(np.asarray(devices), ("core",))
    jit_kwargs = dict(keep_unused=True)
    if donate:
        jit_kwargs["donate_argnums"] = tuple(range(n_params, n_params + n_outs))
    sharded = jax.jit(
        shard_map(_body, mesh=mesh,
                  in_specs=(PartitionSpec("core"),) * (n_params + n_outs),
                  out_specs=(PartitionSpec("core"),) * n_outs,
                  check_rep=False),
        **jit_kwargs)

    def host_args(in_maps):
        concat_in = [
            np.concatenate([np.asarray(m[name]) for m in in_maps], axis=0)
            for name in in_names]
        concat_zeros = [
            np.zeros((N_CORES * a.shape[0],) + tuple(a.shape[1:]), a.dtype)
            for a in out_avals]
        return concat_in + concat_zeros

    def device_args(in_maps):
        from jax.sharding import NamedSharding
        args = host_args(in_maps)
        return [
            jax.device_put(a, NamedSharding(
                mesh, PartitionSpec("core", *(None,) * (a.ndim - 1))))
            for a in args]

    def run(in_maps, args=None):
        if args is None:
            args = host_args(in_maps)
        out_arrs = sharded(*args)
        return [
            {name: np.asarray(out_arrs[i]).reshape(
                (N_CORES,) + tuple(out_avals[i].shape))[c]
             for i, name in enumerate(out_names)}
            for c in range(N_CORES)]

    run.in_names = in_names
    run.out_names = out_names
    run.out_avals = out_avals
    run.sharded = sharded
    run.mesh = mesh
    run.host_args = host_args
    run.device_args = device_args
    return run


def shard_inputs(inputs):
    """Full problem inputs -> 8 per-core input maps (host-side prep)."""
    x = np.asarray(inputs["x"], dtype=np.float32)
    Wq = np.asarray(inputs["Wq"], dtype=np.float32)
    Wk = np.asarray(inputs["Wk"], dtype=np.float32)
    Wv = np.asarray(inputs["Wv"], dtype=np.float32)
    Wo = np.asarray(inputs["Wo"], dtype=np.float32)
    bq = np.asarray(inputs["bq"], dtype=np.float32)
    bk = np.asarray(inputs["bk"], dtype=np.float32)
    in_maps = []
    for b in range(B):
        xt_b = np.ascontiguousarray(x[b].T).astype(BF16)
        for hg in range(2):
            sl = slice(hg * DS, (hg + 1) * DS)
            in_maps.append({
                "xt": xt_b,
                "wq": np.ascontiguousarray(Wq[:, sl]).astype(BF16),
                "wk": np.ascontiguousarray(Wk[:, sl]).astype(BF16),
                "wv": np.ascontiguousarray(Wv[:, sl]).astype(BF16),
                "wo": np.ascontiguousarray(Wo[sl, :]).astype(BF16),
                "bq": np.ascontiguousarray(bq[sl]),
                "bk": np.ascontiguousarray(bk[sl]),
            })
    return in_maps


def gather_output(results, inputs):
    Wo = np.asarray(inputs["Wo"], dtype=np.float32)
    bv = np.asarray(inputs["bv"], dtype=np.float32)
    bo = np.asarray(inputs["bo"], dtype=np.float32)
    bias = bo + bv @ Wo  # V-bias passes through softmax (rows of P sum to 1)
    out = np.empty((B, S, D), dtype=np.float32)
    for b in range(B):
        acc = results[2 * b]["yt"] + results[2 * b + 1]["yt"]  # [D, S]
        out[b] = acc.T + bias
    return out


_CACHE = {}


def _get_runner():
    if "runner" not in _CACHE:
        nc = build_module()
        _CACHE["nc"] = nc
        _CACHE["runner"] = make_runner(nc)
    return _CACHE["runner"]


def kernel(**inputs) -> np.ndarray:
    runner = _get_runner()
    in_maps = shard_inputs(inputs)
    results = runner(in_maps)
    return gather_output(results, inputs)



# revision 26
# speedup vs baseline: 1.1347x; 1.0017x over previous
"""Multi-headed attention on 8 TRN2 NeuronCores (Bass/Tile).

Problem: x[4, 2048, 1024] f32; 16 heads, Dk=64.
  Q = x@Wq+bq, K = x@Wk+bk, V = x@Wv+bv  (per-head split)
  out = softmax(QK^T/8) V  re-merged, @Wo + bo

Sharding (tensor-parallel heads x batch): core = b*2 + hg
  b  in 0..3  : batch index
  hg in 0..1  : head group (8 heads = 512 of the 1024 d_model dims)
Each core gets x[b]^T (pre-transposed on host, bf16) and the hg-slice of the
weights, and produces the partial Y^T = (P V_hg) @ Wo_hg  (d-major, f32,
no biases). Host sums the two head-group partials per batch, transposes, and
adds bo + bv@Wo (the V-bias commutes through softmax: rows of P sum to 1).

On-core dataflow (all matmul operands bf16, PSUM f32):
  Xt   [1024,2048] d-major input (host-provided)
  Qt,Kt[512,2048]  d-major projections; bias added during PSUM->SBUF copy
  Vaug [2048, 8,65] natural V with a ones column per head (rowsum trick)
  per (q-block 512, head-pair): St^T [128k,2x512q] psum  (2 heads packed in
     the 128-row PE array via base-partition 0/64 row tiling, K=64 each)
  P~ = exp(St^T / 8) -> bf16 (one ACT op per [128,1024] tile; no max-sub:
     |scores| <~ 2 for this problem's distribution, exp is safe in f32)
  O^T+rowsum = [V_h | 1]^T @ P~^T  accumulated over 16 k-chunks -> [65, 512]
  Ot = O^T * (1/rowsum broadcast)  -> bf16  (odd heads DMA-shifted to
     partitions 64..127 so the final matmul sees full 128-row d-chunks)
  Y^T = Wo^T @ Ot  accumulated over 4 d-chunks -> f32 -> DRAM
"""

import os
import numpy as np
import ml_dtypes
from contextlib import ExitStack

import jax
from jax.sharding import Mesh, PartitionSpec
from jax.experimental.shard_map import shard_map

import concourse.bass as bass
import concourse.tile as tile
from concourse import bacc, mybir
from concourse import bass2jax

BF16 = ml_dtypes.bfloat16

B, S, D, H, DK = 4, 2048, 1024, 16, 64
HPG = 8              # heads per group (per core)
DS = HPG * DK        # 512: d_model slice per core
N_CORES = 8
P = 128
QW = 512             # q block width
QB = S // QW         # 4 q blocks
KC = D // P          # 8 contraction chunks for projections
DC = DS // P         # 4 d-chunks of the head-group slice (= head pairs)
TC = S // P          # 16 token chunks (= k_tok chunks)
FP32 = mybir.dt.float32
BF = mybir.dt.bfloat16
F8 = mybir.dt.float8e4
F16 = mybir.dt.float16
AF = mybir.ActivationFunctionType


# ablation switches for performance bisection (all True = full kernel)
ABLATE = {"exp": True, "pv": True, "norm": True, "final": True}


def build_tile_kernel(ctx: ExitStack, tc_ctx: tile.TileContext,
                      xt, wq, wk, wv, wo, bq, bk, yt, repeat=1):
    nc = tc_ctx.nc
    tc = tc_ctx

    wpool = ctx.enter_context(tc.tile_pool(name="w", bufs=1))
    xpool = ctx.enter_context(tc.tile_pool(name="x", bufs=1))
    qkpool = ctx.enter_context(tc.tile_pool(name="qk", bufs=1))
    vpool = ctx.enter_context(tc.tile_pool(name="v", bufs=1))
    opool = ctx.enter_context(tc.tile_pool(name="o", bufs=1))
    ptpool = ctx.enter_context(tc.tile_pool(name="pt", bufs=16))
    small = ctx.enter_context(tc.tile_pool(name="small", bufs=3))
    nrm = ctx.enter_context(tc.tile_pool(name="nrm", bufs=2))
    ypool = ctx.enter_context(tc.tile_pool(name="y", bufs=3))
    psA = ctx.enter_context(tc.tile_pool(name="psA", bufs=2, space="PSUM"))
    psB = ctx.enter_context(tc.tile_pool(name="psB", bufs=4, space="PSUM"))
    dscr = ctx.enter_context(tc.tile_pool(name="dscr", bufs=4, space="DRAM"))

    # ---- inputs -> SBUF, ordered so the first attention unit's operands
    # land first: x token-block 0 + the c=0 column slices of Wq/Wk gate the
    # first projections and therefore the start of the exp stream ----
    # batched: one dma_start per logical block (the SP sequencer costs
    # ~565ns per call; fine-grained per-chunk DMAs serialized ~14us of
    # issue time ahead of the first projection)
    w_q = wpool.tile([P, KC, DS], BF)
    w_k = wpool.tile([P, KC, DS], BF)
    w_v = wpool.tile([P, KC, DS], BF)
    x_sb = xpool.tile([P, KC, S], BF)
    nc.sync.dma_start(x_sb[:, :, 0:QW],
                      xt.rearrange("(c p) q -> p c q", p=P)[:, :, 0:QW])
    nc.sync.dma_start(w_q[:, :, 0:P],
                      wq.rearrange("(c p) d -> p c d", p=P)[:, :, 0:P])
    nc.sync.dma_start(w_k[:, :, 0:P],
                      wk.rearrange("(c p) d -> p c d", p=P)[:, :, 0:P])
    bq_sb = wpool.tile([P, DC], FP32)
    bk_sb = wpool.tile([P, DC], FP32)
    nc.sync.dma_start(bq_sb[:], bq.rearrange("(c p) -> p c", p=P))
    nc.sync.dma_start(bk_sb[:], bk.rearrange("(c p) -> p c", p=P))
    for tb in range(1, QB):
        nc.sync.dma_start(
            x_sb[:, :, tb * QW:(tb + 1) * QW],
            xt.rearrange("(c p) q -> p c q", p=P)[:, :, tb * QW:(tb + 1) * QW])
    nc.sync.dma_start(w_q[:, :, P:DS],
                      wq.rearrange("(c p) d -> p c d", p=P)[:, :, P:DS])
    nc.sync.dma_start(w_k[:, :, P:DS],
                      wk.rearrange("(c p) d -> p c d", p=P)[:, :, P:DS])
    nc.sync.dma_start(w_v[:], wv.rearrange("(c p) d -> p c d", p=P))
    w_o = wpool.tile([P, DC, D], BF)
    nc.sync.dma_start(w_o[:], wo.rearrange("(c p) d -> p c d", p=P))

    qt = qkpool.tile([P, DC, S], BF)
    kt = qkpool.tile([P, DC, S], BF)
    # V for fp8 DoubleRow PV: [double-chunk, parity, j, k-plane, 128 cols].
    # Head 2j+parity: even heads keep V at cols 0:64 with the rowsum ones
    # column at 64; odd heads put V at cols 64:128 (their ot rows) with ones
    # at col 0.  Rowsums then appear at psum rows 64 / 0, both reachable by
    # the on-chip 32x32-block transpose normalization, and each head's O
    # rows coincide with its ot rows so the normalize mults never shift
    # partitions.  Zeros elsewhere keep the junk psum rows finite.
    # k-token = 256*c + 128*plane + p; each (c, parity, j) slice is a
    # contiguous [2, 128] block (dual-fp8 ldweights needs plane-stride ==
    # column count == 128).
    vaug = vpool.tile([P, TC // 2, 2, DC, 2, P], F8)
    ot = opool.tile([P, DC, S], BF)

    nc.vector.memset(vaug[:], 0.0)
    nc.vector.memset(vaug[:, :, 0, :, :, DK], 1.0)
    nc.vector.memset(vaug[:, :, 1, :, :, 0], 1.0)
    # all-ones lhsT rows for the normalize broadcast matmuls (engine APs may
    # only start at partitions 0/32/64/96, so the two recip rows live at
    # partitions 0 and 32 and each gets its own K=1 matmul)
    E_sb = wpool.tile([P, DK], F16)
    nc.vector.memset(E_sb[0:1, :], 1.0)
    nc.vector.memset(E_sb[32:33, :], 1.0)

    def qk_proj_chunk(c):
        """Project d_out chunk c of Q^T and K^T (128 dims x all 2048 tokens).
        tb-outer so the first token block's Q AND K finish first: the first
        scores tile needs Q(tb0) + K(tb0) only."""
        for tb in range(QB):
            for w_sb, b_sb, dest in ((w_q, bq_sb, qt), (w_k, bk_sb, kt)):
                ps = psB.tile([P, QW], FP32, tag="b")
                for kc in range(KC):
                    nc.tensor.matmul(
                        ps[:],
                        lhsT=w_sb[:, kc, c * P:(c + 1) * P],
                        rhs=x_sb[:, kc, tb * QW:(tb + 1) * QW],
                        start=(kc == 0), stop=(kc == KC - 1))
                nc.vector.tensor_scalar_add(
                    dest[:, c, tb * QW:(tb + 1) * QW], ps[:], b_sb[:, c:c + 1])

    def v_proj():
        for tci in range(TC):
            ps = psB.tile([P, DS], FP32, tag="b")
            for kc in range(KC):
                nc.tensor.matmul(
                    ps[:],
                    lhsT=x_sb[:, kc, tci * P:(tci + 1) * P],
                    rhs=w_v[:, kc, :],
                    start=(kc == 0), stop=(kc == KC - 1))
            vsrc = ps.rearrange("p (j two e) -> p two j e", two=2, e=DK)
            nc.vector.tensor_copy(
                vaug[:, tci // 2, 0, :, tci % 2, 0:DK], vsrc[:, 0])
            nc.vector.tensor_copy(
                vaug[:, tci // 2, 1, :, tci % 2, DK:P], vsrc[:, 1])

    unit_pts = {}

    def scores_exp(qb, j):
        """scores + exp for head pair j, q block qb."""
        pts = []
        for kc2 in range(TC):
            psS = psA.tile([P, 2 * QW], FP32, tag="s")
            for h01 in range(2):
                lo = h01 * DK
                nc.tensor.matmul(
                    psS[:, h01 * QW:(h01 + 1) * QW],
                    lhsT=kt[lo:lo + DK, j, kc2 * P:(kc2 + 1) * P],
                    rhs=qt[lo:lo + DK, j, qb * QW:(qb + 1) * QW],
                    start=True, stop=True)
            if ABLATE["exp"]:
                # fp8 P~, two 128-token k-planes per tile (DoubleRow layout);
                # [head, plane, q] so each head's [2, 512] planes-pair is
                # contiguous for the PV moving operand
                if kc2 % 2 == 0:
                    pt = ptpool.tile([P, 2, 2, QW], F8, tag="pt")
                    pts.append(pt)
                nc.scalar.activation(pts[-1][:, :, kc2 % 2, :], psS[:],
                                     AF.Exp, scale=0.125)
        unit_pts[(qb, j)] = pts

    def pv_norm(qb, j):
        """PV + normalize for head pair j, q block qb."""
        if not (ABLATE["exp"] and ABLATE["pv"]):
            return
        pts = unit_pts.pop((qb, j))
        # both heads' PV chains interleaved per double-k-chunk: fp8 DoubleRow
        # matmuls (2x bf16 rate); each P~ tile is fully consumed at its own
        # k-step and the accumulations overlap on PE
        psOs = [psB.tile([P, QW], FP32, tag="b", name=f"psO{_h}")
                for _h in range(2)]
        # one accumulation group per head tile: start/stop only at the global
        # first/last matmul (psum "zero regions" are 2KB = the whole tile; a
        # second start would lazily re-zero the sibling slab's partial sums)
        for c in range(TC // 2):
            for h01 in range(2):
                for sl in range(2):
                    nc.tensor.matmul(
                        psOs[h01][:, sl * 256:(sl + 1) * 256],
                        lhsT=vaug[:, c, h01, j, :, :],
                        rhs=pts[c][:, h01, :, sl * 256:(sl + 1) * 256],
                        perf_mode=mybir.MatmulPerfMode.DoubleRow,
                        start=(c == 0 and sl == 0),
                        stop=(c == TC // 2 - 1 and sl == 1))
        qcols = slice(qb * QW, (qb + 1) * QW)
        if not ABLATE["norm"]:
            # timing-ablation path: skip normalization, copy raw O
            # (partition-preserving; wrong results, right timing shape)
            nc.vector.tensor_copy(ot[0:DK, j, qcols], psOs[0][0:DK, :])
            nc.vector.tensor_copy(ot[DK:P, j, qcols], psOs[1][DK:P, :])
            return
        # quick copies of O+rowsum to SBUF release the PSUM slots (the
        # partition count doesn't affect DVE cost, only free size does)
        ou0 = small.tile([P, QW], FP32, tag="ou0")
        ou1 = small.tile([P, QW], FP32, tag="ou1")
        nc.vector.tensor_copy(ou0[0:DK + 1, :], psOs[0][0:DK + 1, :])
        nc.vector.tensor_copy(ou1[:], psOs[1][:])
        # on-chip 1/rowsum broadcast (no DRAM hops):
        #  - gather both rowsum rows onto partitions 0/1 of one tile
        #    (small sbuf->sbuf DMAs on the idle gpsimd queue)
        #  - DVE 32x32-block transpose puts them on columns =0/1 (mod 32)
        #  - reciprocal over just those columns (fp32 -> fp16)
        #  - transpose back: rows 0/1 = 1/rowsum vectors
        #  - one fp16 K=2 matmul broadcasts them to 64 psum rows each
        rrow = nrm.tile([P, QW], FP32, tag="rrow")
        nc.vector.memset(rrow[0:DK, :], 1.0)  # keep transpose junk finite
        nc.gpsimd.dma_start(rrow[0:1, :], ou0[DK:DK + 1, :])
        nc.gpsimd.dma_start(rrow[32:33, :], ou1[0:1, :])
        tc_t = nrm.tile([P, QW], FP32, tag="tc")
        nc.vector.transpose(tc_t[0:DK, :], rrow[0:DK, :])
        tc2 = nrm.tile([P, QW], F16, tag="tc2")
        nc.vector.memset(tc2[0:DK, :], 1.0)
        with nc.allow_low_precision(reason="1/rowsum in fp16: 5e-4 rel, "
                                    "well inside the error budget"):
            nc.vector.reciprocal(
                tc2[0:DK].rearrange("p (b t) -> p b t", t=32)[:, :, 0:1],
                tc_t[0:DK].rearrange("p (b t) -> p b t", t=32)[:, :, 0:1])
        trec = nrm.tile([P, QW], F16, tag="trec")
        nc.vector.transpose(trec[0:DK, :], tc2[0:DK, :])
        bc_ps = psB.tile([P, QW], FP32, tag="b", name="bc")
        nc.tensor.matmul(bc_ps[0:DK, :], lhsT=E_sb[0:1, :], rhs=trec[0:1, :],
                         start=True, stop=True)
        nc.tensor.matmul(bc_ps[DK:P, :], lhsT=E_sb[32:33, :],
                         rhs=trec[32:33, :], start=True, stop=True)
        nc.vector.tensor_mul(ot[0:DK, j, qcols], ou0[0:DK, :], bc_ps[0:DK, :])
        nc.vector.tensor_mul(ot[DK:P, j, qcols], ou1[DK:P, :], bc_ps[DK:P, :])

    def final_qb(qb):
        for oc in range(D // P):
            ps = psB.tile([P, QW], FP32, tag="b")
            for dc in range(DC):
                nc.tensor.matmul(
                    ps[:],
                    lhsT=w_o[:, dc, oc * P:(oc + 1) * P],
                    rhs=ot[:, dc, qb * QW:(qb + 1) * QW],
                    start=(dc == 0), stop=(dc == DC - 1))
            y_sb = ypool.tile([P, QW], FP32, tag="y")
            # explicit DVE: finals now run alongside exps, and nc.any would
            # put these copies on the exp-critical ACT engine
            nc.vector.tensor_copy(y_sb[:], ps[:])
            nc.sync.dma_start(
                yt[oc * P:(oc + 1) * P, qb * QW:(qb + 1) * QW], y_sb[:])

    y012 = opool.tile([P, D // P, QW], FP32)

    def final_qb_partial(qb):
        """d-chunks 0..2 of the last q-block's output projection, runnable
        before the last attention unit: shrinks the tail to one matmul."""
        for oc in range(D // P):
            ps = psB.tile([P, QW], FP32, tag="b")
            for dc in range(DC - 1):
                nc.tensor.matmul(
                    ps[:],
                    lhsT=w_o[:, dc, oc * P:(oc + 1) * P],
                    rhs=ot[:, dc, qb * QW:(qb + 1) * QW],
                    start=(dc == 0), stop=(dc == DC - 2))
            nc.vector.tensor_copy(y012[:, oc, :], ps[:])

    def final_qb_tail(qb):
        for oc in range(D // P):
            ps = psB.tile([P, QW], FP32, tag="b")
            nc.tensor.matmul(
                ps[:],
                lhsT=w_o[:, DC - 1, oc * P:(oc + 1) * P],
                rhs=ot[:, DC - 1, qb * QW:(qb + 1) * QW],
                start=True, stop=True)
            y_sb = ypool.tile([P, QW], FP32, tag="y")
            nc.vector.tensor_add(y_sb[:], ps[:], y012[:, oc, :])
            nc.sync.dma_start(
                yt[oc * P:(oc + 1) * P, qb * QW:(qb + 1) * QW], y_sb[:])

    def compute_once():
        # Emission order == scheduler priority.  Software-pipelined: the
        # scores+exp of unit u+1 outrank the PV of unit u, so the ACT exp
        # stream (the co-bottleneck) never waits for PE catch-up work;
        # PV/projections/finals fill PE slack between paced scores tiles.
        # j-major spreads the Q/K projection chunks across the kernel (each
        # chunk c is consumed by 4 consecutive units) instead of packing all
        # four into q-block 0's window; finals land inside head-pair group 3,
        # one per unit, with qb=3's final split so the tail after the last
        # exp is just one accumulation step.
        units = [(qb, j) for j in range(DC) for qb in range(QB)]
        qk_proj_chunk(0)
        scores_exp(*units[0])
        v_proj()
        qk_proj_chunk(1)
        for i, (qb, j) in enumerate(units):
            if i + 1 < len(units):
                scores_exp(*units[i + 1])
            if i == 1:
                qk_proj_chunk(2)
            elif i == 5:
                qk_proj_chunk(3)
            elif i == 12 and ABLATE["final"]:
                final_qb_partial(3)
            pv_norm(qb, j)
            if j == DC - 1 and ABLATE["final"]:
                if qb == QB - 1:
                    final_qb_tail(qb)
                else:
                    final_qb(qb)

    for _ in range(repeat):
        compute_once()


def build_module(repeat=1):
    nc = bacc.Bacc("TRN2", target_bir_lowering=False, debug=False)
    xt = nc.dram_tensor("xt", [D, S], BF, kind="ExternalInput").ap()
    wq = nc.dram_tensor("wq", [D, DS], BF, kind="ExternalInput").ap()
    wk = nc.dram_tensor("wk", [D, DS], BF, kind="ExternalInput").ap()
    wv = nc.dram_tensor("wv", [D, DS], BF, kind="ExternalInput").ap()
    wo = nc.dram_tensor("wo", [DS, D], BF, kind="ExternalInput").ap()
    bq = nc.dram_tensor("bq", [DS], FP32, kind="ExternalInput").ap()
    bk = nc.dram_tensor("bk", [DS], FP32, kind="ExternalInput").ap()
    yt = nc.dram_tensor("yt", [D, S], FP32, kind="ExternalOutput").ap()
    with tile.TileContext(nc) as tc:
        with ExitStack() as ctx:
            build_tile_kernel(ctx, tc, xt, wq, wk, wv, wo, bq, bk, yt,
                              repeat=repeat)
    nc.compile()
    return nc


def _collect_io(nc):
    partition_name = (nc.partition_id_tensor.name
                      if nc.partition_id_tensor else None)
    in_names, out_names, out_avals = [], [], []
    for alloc in nc.m.functions[0].allocations:
        if not isinstance(alloc, mybir.MemoryLocationSet):
            continue
        name = alloc.memorylocations[0].name
        if alloc.kind == "ExternalInput":
            if name != partition_name:
                in_names.append(name)
        elif alloc.kind == "ExternalOutput":
            out_names.append(name)
            out_avals.append(jax.core.ShapedArray(
                tuple(alloc.tensor_shape), mybir.dt.np(alloc.dtype)))
    return in_names, out_names, out_avals, partition_name


def make_runner(nc, donate=False):
    """Multi-core PJRT runner (the run_bass_via_pjrt path, but with the
    jitted executable retained so repeated calls don't re-lower).

    donate=False: the kernel writes every element of its outputs, so the
    zero output-operands never need to be donated; keeping them allows the
    same device-resident args to be re-used for repeated timed calls."""
    bass2jax.install_neuronx_cc_hook()
    in_names, out_names, out_avals, partition_name = _collect_io(nc)
    n_params, n_outs = len(in_names), len(out_names)
    all_names = in_names + out_names
    if partition_name is not None:
        all_names = all_names + [partition_name]

    def _body(*args):
        operands = list(args)
        if partition_name is not None:
            operands.append(bass2jax.partition_id_tensor())
        outs = bass2jax._bass_exec_p.bind(
            *operands,
            out_avals=tuple(out_avals),
            in_names=tuple(all_names),
            out_names=tuple(out_names),
            lowering_input_output_aliases=(),
            sim_require_finite=True,
            sim_require_nnan=True,
            nc=nc,
        )
        return tuple(outs)

    devices = jax.devices()[:N_CORES]
    mesh = Mesh(np.asarray(devices), ("core",))
    jit_kwargs = dict(keep_unused=True)
    if donate:
        jit_kwargs["donate_argnums"] = tuple(range(n_params, n_params + n_outs))
    sharded = jax.jit(
        shard_map(_body, mesh=mesh,
                  in_specs=(PartitionSpec("core"),) * (n_params + n_outs),
                  out_specs=(PartitionSpec("core"),) * n_outs,
                  check_rep=False),
        **jit_kwargs)

    def host_args(in_maps):
        concat_in = [
            np.concatenate([np.asarray(m[name]) for m in in_maps], axis=0)
            for name in in_names]
        concat_zeros = [
            np.zeros((N_CORES * a.shape[0],) + tuple(a.shape[1:]), a.dtype)
            for a in out_avals]
        return concat_in + concat_zeros

    def device_args(in_maps):
        from jax.sharding import NamedSharding
        args = host_args(in_maps)
        return [
            jax.device_put(a, NamedSharding(
                mesh, PartitionSpec("core", *(None,) * (a.ndim - 1))))
            for a in args]

    def run(in_maps, args=None):
        if args is None:
            args = host_args(in_maps)
        out_arrs = sharded(*args)
        return [
            {name: np.asarray(out_arrs[i]).reshape(
                (N_CORES,) + tuple(out_avals[i].shape))[c]
             for i, name in enumerate(out_names)}
            for c in range(N_CORES)]

    run.in_names = in_names
    run.out_names = out_names
    run.out_avals = out_avals
    run.sharded = sharded
    run.mesh = mesh
    run.host_args = host_args
    run.device_args = device_args
    return run


def shard_inputs(inputs):
    """Full problem inputs -> 8 per-core input maps (host-side prep)."""
    x = np.asarray(inputs["x"], dtype=np.float32)
    Wq = np.asarray(inputs["Wq"], dtype=np.float32)
    Wk = np.asarray(inputs["Wk"], dtype=np.float32)
    Wv = np.asarray(inputs["Wv"], dtype=np.float32)
    Wo = np.asarray(inputs["Wo"], dtype=np.float32)
    bq = np.asarray(inputs["bq"], dtype=np.float32)
    bk = np.asarray(inputs["bk"], dtype=np.float32)
    in_maps = []
    for b in range(B):
        xt_b = np.ascontiguousarray(x[b].T).astype(BF16)
        for hg in range(2):
            sl = slice(hg * DS, (hg + 1) * DS)
            in_maps.append({
                "xt": xt_b,
                "wq": np.ascontiguousarray(Wq[:, sl]).astype(BF16),
                "wk": np.ascontiguousarray(Wk[:, sl]).astype(BF16),
                "wv": np.ascontiguousarray(Wv[:, sl]).astype(BF16),
                "wo": np.ascontiguousarray(Wo[sl, :]).astype(BF16),
                "bq": np.ascontiguousarray(bq[sl]),
                "bk": np.ascontiguousarray(bk[sl]),
            })
    return in_maps


def gather_output(results, inputs):
    Wo = np.asarray(inputs["Wo"], dtype=np.float32)
    bv = np.asarray(inputs["bv"], dtype=np.float32)
    bo = np.asarray(inputs["bo"], dtype=np.float32)
    bias = bo + bv @ Wo  # V-bias passes through softmax (rows of P sum to 1)
    out = np.empty((B, S, D), dtype=np.float32)
    for b in range(B):
        acc = results[2 * b]["yt"] + results[2 * b + 1]["yt"]  # [D, S]
        out[b] = acc.T + bias
    return out


_CACHE = {}


def _get_runner():
    if "runner" not in _CACHE:
        nc = build_module()
        _CACHE["nc"] = nc
        _CACHE["runner"] = make_runner(nc)
    return _CACHE["runner"]


def kernel(**inputs) -> np.ndarray:
    runner = _get_runner()
    in_maps = shard_inputs(inputs)
    results = runner(in_maps)
    return gather_output(results, inputs)



# revision 36
# speedup vs baseline: 1.1889x; 1.0478x over previous
"""Multi-headed attention on 8 TRN2 NeuronCores (Bass/Tile).

Problem: x[4, 2048, 1024] f32; 16 heads, Dk=64.
  Q = x@Wq+bq, K = x@Wk+bk, V = x@Wv+bv  (per-head split)
  out = softmax(QK^T/8) V  re-merged, @Wo + bo

Sharding (tensor-parallel heads x batch): core = b*2 + hg
  b  in 0..3  : batch index
  hg in 0..1  : head group (8 heads = 512 of the 1024 d_model dims)
Each core gets x[b]^T (pre-transposed on host, bf16) and the hg-slice of the
weights, and produces the partial Y^T = (P V_hg) @ Wo_hg  (d-major, f32,
no biases). Host sums the two head-group partials per batch, transposes, and
adds bo + bv@Wo (the V-bias commutes through softmax: rows of P sum to 1).

On-core dataflow (all matmul operands bf16, PSUM f32):
  Xt   [1024,2048] d-major input (host-provided)
  Qt,Kt[512,2048]  d-major projections; bias added during PSUM->SBUF copy
  Vaug [2048, 8,65] natural V with a ones column per head (rowsum trick)
  per (q-block 512, head-pair): St^T [128k,2x512q] psum  (2 heads packed in
     the 128-row PE array via base-partition 0/64 row tiling, K=64 each)
  P~ = exp(St^T / 8) -> bf16 (one ACT op per [128,1024] tile; no max-sub:
     |scores| <~ 2 for this problem's distribution, exp is safe in f32)
  O^T+rowsum = [V_h | 1]^T @ P~^T  accumulated over 16 k-chunks -> [65, 512]
  Ot = O^T * (1/rowsum broadcast)  -> bf16  (odd heads DMA-shifted to
     partitions 64..127 so the final matmul sees full 128-row d-chunks)
  Y^T = Wo^T @ Ot  accumulated over 4 d-chunks -> f32 -> DRAM
"""

import os
import numpy as np
import ml_dtypes
from contextlib import ExitStack

import jax
from jax.sharding import Mesh, PartitionSpec
from jax.experimental.shard_map import shard_map

import concourse.bass as bass
import concourse.tile as tile
from concourse import bacc, mybir
from concourse import bass2jax

BF16 = ml_dtypes.bfloat16

B, S, D, H, DK = 4, 2048, 1024, 16, 64
HPG = 8              # heads per group (per core)
DS = HPG * DK        # 512: d_model slice per core
N_CORES = 8
P = 128
QW = 512             # q block width
QB = S // QW         # 4 q blocks
KC = D // P          # 8 contraction chunks for projections
DC = DS // P         # 4 d-chunks of the head-group slice (= head pairs)
TC = S // P          # 16 token chunks (= k_tok chunks)
FP32 = mybir.dt.float32
BF = mybir.dt.bfloat16
F8 = mybir.dt.float8e4
F16 = mybir.dt.float16
AF = mybir.ActivationFunctionType


# ablation switches for performance bisection (all True = full kernel)
ABLATE = {"exp": True, "pv": True, "norm": True, "final": True}


def build_tile_kernel(ctx: ExitStack, tc_ctx: tile.TileContext,
                      xt, wq, wk, wv, wo, bq, bk, yt, repeat=1):
    nc = tc_ctx.nc
    tc = tc_ctx

    wpool = ctx.enter_context(tc.tile_pool(name="w", bufs=1))
    xpool = ctx.enter_context(tc.tile_pool(name="x", bufs=1))
    qkpool = ctx.enter_context(tc.tile_pool(name="qk", bufs=1))
    vpool = ctx.enter_context(tc.tile_pool(name="v", bufs=1))
    opool = ctx.enter_context(tc.tile_pool(name="o", bufs=1))
    ptpool = ctx.enter_context(tc.tile_pool(name="pt", bufs=16))
    small = ctx.enter_context(tc.tile_pool(name="small", bufs=3))
    nrm = ctx.enter_context(tc.tile_pool(name="nrm", bufs=2))
    ypool = ctx.enter_context(tc.tile_pool(name="y", bufs=3))
    psA = ctx.enter_context(tc.tile_pool(name="psA", bufs=2, space="PSUM"))
    psB = ctx.enter_context(tc.tile_pool(name="psB", bufs=4, space="PSUM"))
    dscr = ctx.enter_context(tc.tile_pool(name="dscr", bufs=4, space="DRAM"))

    # ---- inputs -> SBUF, ordered so the first attention unit's operands
    # land first: x token-block 0 + the c=0 column slices of Wq/Wk gate the
    # first projections and therefore the start of the exp stream ----
    # batched: one dma_start per logical block (the SP sequencer costs
    # ~565ns per call; fine-grained per-chunk DMAs serialized ~14us of
    # issue time ahead of the first projection)
    w_q = wpool.tile([P, KC, DS], BF)
    w_k = wpool.tile([P, KC, DS], BF)
    w_v = wpool.tile([P, KC, DS], BF)
    x_sb = xpool.tile([P, KC, S], BF)
    nc.sync.dma_start(w_q[:, :, 0:P],
                      wq.rearrange("(c p) d -> p c d", p=P)[:, :, 0:P])
    nc.sync.dma_start(x_sb[:, :, 0:QW],
                      xt.rearrange("(c p) q -> p c q", p=P)[:, :, 0:QW])
    nc.sync.dma_start(w_k[:, :, 0:P],
                      wk.rearrange("(c p) d -> p c d", p=P)[:, :, 0:P])
    bq_sb = wpool.tile([P, DC], FP32)
    bk_sb = wpool.tile([P, DC], FP32)
    nc.sync.dma_start(bq_sb[:], bq.rearrange("(c p) -> p c", p=P))
    nc.sync.dma_start(bk_sb[:], bk.rearrange("(c p) -> p c", p=P))
    for tb in range(1, QB):
        nc.sync.dma_start(
            x_sb[:, :, tb * QW:(tb + 1) * QW],
            xt.rearrange("(c p) q -> p c q", p=P)[:, :, tb * QW:(tb + 1) * QW])
    nc.sync.dma_start(w_q[:, :, P:DS],
                      wq.rearrange("(c p) d -> p c d", p=P)[:, :, P:DS])
    nc.sync.dma_start(w_k[:, :, P:DS],
                      wk.rearrange("(c p) d -> p c d", p=P)[:, :, P:DS])
    nc.sync.dma_start(w_v[:], wv.rearrange("(c p) d -> p c d", p=P))
    w_o = wpool.tile([P, DC, D], BF)
    nc.sync.dma_start(w_o[:], wo.rearrange("(c p) d -> p c d", p=P))

    qt = qkpool.tile([P, DC, S], BF)
    kt = qkpool.tile([P, DC, S], BF)
    # V for fp8 DoubleRow PV: [double-chunk, parity, j, k-plane, 128 cols].
    # Head 2j+parity: even heads keep V at cols 0:64 with the rowsum ones
    # column at 64; odd heads put V at cols 64:128 (their ot rows) with ones
    # at col 0.  Rowsums then appear at psum rows 64 / 0, both reachable by
    # the on-chip 32x32-block transpose normalization, and each head's O
    # rows coincide with its ot rows so the normalize mults never shift
    # partitions.  Zeros elsewhere keep the junk psum rows finite.
    # k-token = 256*c + 128*plane + p; each (c, parity, j) slice is a
    # contiguous [2, 128] block (dual-fp8 ldweights needs plane-stride ==
    # column count == 128).
    vaug = vpool.tile([P, TC // 2, 2, DC, 2, P], F8)
    ot = opool.tile([P, DC, S], BF)

    # on gpsimd: the 16K-element zero fill would cost ~17us of DVE right
    # when the first Q/K bias-adds need it; Pool is otherwise idle
    nc.gpsimd.memset(vaug[:], 0.0)
    nc.gpsimd.memset(vaug[:, :, 0, :, :, DK], 1.0)
    nc.gpsimd.memset(vaug[:, :, 1, :, :, 0], 1.0)
    # all-ones lhsT rows for the normalize broadcast matmuls (engine APs may
    # only start at partitions 0/32/64/96; the two recip rows live at
    # partitions 64 (head 2j) and 0 (head 2j+1) and each gets its own K=1
    # matmul with tile_position (64,0) / (0,64))
    E_sb = wpool.tile([P, DK], F16)
    nc.vector.memset(E_sb[0:1, :], 1.0)
    nc.vector.memset(E_sb[DK:DK + 1, :], 1.0)
    # persistent double-buffered recip tile: initialized once so the
    # back-transpose may read the never-written columns (pool tiles would be
    # "fresh" uninitialized tensors every unit)
    tc2P = wpool.tile([P, 2, QW], F16)
    nc.gpsimd.memset(tc2P[:], 1.0)

    def qk_proj_tb(c, tb):
        """Project d_out chunk c of Q^T and K^T for token block tb."""
        for w_sb, b_sb, dest in ((w_q, bq_sb, qt), (w_k, bk_sb, kt)):
            ps = psB.tile([P, QW], FP32, tag="b")
            for kc in range(KC):
                nc.tensor.matmul(
                    ps[:],
                    lhsT=w_sb[:, kc, c * P:(c + 1) * P],
                    rhs=x_sb[:, kc, tb * QW:(tb + 1) * QW],
                    start=(kc == 0), stop=(kc == KC - 1))
            nc.vector.tensor_scalar_add(
                dest[:, c, tb * QW:(tb + 1) * QW], ps[:], b_sb[:, c:c + 1])

    def qk_proj_chunk(c):
        for tb in range(QB):
            qk_proj_tb(c, tb)

    def v_proj():
        for tci in range(TC):
            ps = psB.tile([P, DS], FP32, tag="b")
            for kc in range(KC):
                nc.tensor.matmul(
                    ps[:],
                    lhsT=x_sb[:, kc, tci * P:(tci + 1) * P],
                    rhs=w_v[:, kc, :],
                    start=(kc == 0), stop=(kc == KC - 1))
            vsrc = ps.rearrange("p (j two e) -> p two j e", two=2, e=DK)
            nc.vector.tensor_copy(
                vaug[:, tci // 2, 0, :, tci % 2, 0:DK], vsrc[:, 0])
            nc.vector.tensor_copy(
                vaug[:, tci // 2, 1, :, tci % 2, DK:P], vsrc[:, 1])

    unit_pts = {}
    unit_no = [0]

    def scores_exp(qb, j, kc2s=None):
        """scores + exp for head pair j, q block qb."""
        pts = unit_pts.setdefault((qb, j), [])
        for kc2 in (range(TC) if kc2s is None else kc2s):
            psS = psA.tile([P, 2 * QW], FP32, tag="s")
            for h01 in range(2):
                lo = h01 * DK
                nc.tensor.matmul(
                    psS[:, h01 * QW:(h01 + 1) * QW],
                    lhsT=kt[lo:lo + DK, j, kc2 * P:(kc2 + 1) * P],
                    rhs=qt[lo:lo + DK, j, qb * QW:(qb + 1) * QW],
                    start=True, stop=True)
            if ABLATE["exp"]:
                # fp8 P~, two 128-token k-planes per tile (DoubleRow layout);
                # [head, plane, q] so each head's [2, 512] planes-pair is
                # contiguous for the PV moving operand
                if kc2 % 2 == 0:
                    pt = ptpool.tile([P, 2, 2, QW], F8, tag="pt")
                    pts.append(pt)
                nc.scalar.activation(pts[-1][:, :, kc2 % 2, :], psS[:],
                                     AF.Exp, scale=0.125)

    def pv_norm(qb, j):
        """PV + normalize for head pair j, q block qb."""
        if not (ABLATE["exp"] and ABLATE["pv"]):
            return
        pts = unit_pts.pop((qb, j))
        # both heads' PV chains interleaved per double-k-chunk: fp8 DoubleRow
        # matmuls (2x bf16 rate); each P~ tile is fully consumed at its own
        # k-step and the accumulations overlap on PE
        psOs = [psB.tile([P, QW], FP32, tag="b", name=f"psO{_h}")
                for _h in range(2)]
        # one accumulation group per head tile: start/stop only at the global
        # first/last matmul (psum "zero regions" are 2KB = the whole tile; a
        # second start would lazily re-zero the sibling slab's partial sums)
        for c in range(TC // 2):
            for h01 in range(2):
                for sl in range(2):
                    nc.tensor.matmul(
                        psOs[h01][:, sl * 256:(sl + 1) * 256],
                        lhsT=vaug[:, c, h01, j, :, :],
                        rhs=pts[c][:, h01, :, sl * 256:(sl + 1) * 256],
                        perf_mode=mybir.MatmulPerfMode.DoubleRow,
                        start=(c == 0 and sl == 0),
                        stop=(c == TC // 2 - 1 and sl == 1))
        qcols = slice(qb * QW, (qb + 1) * QW)
        if not ABLATE["norm"]:
            # timing-ablation path: skip normalization, copy raw O
            # (partition-preserving; wrong results, right timing shape)
            nc.vector.tensor_copy(ot[0:DK, j, qcols], psOs[0][0:DK, :])
            nc.vector.tensor_copy(ot[DK:P, j, qcols], psOs[1][DK:P, :])
            return
        # quick copies of O+rowsum to SBUF release the PSUM slots (the
        # partition count doesn't affect DVE cost, only free size does)
        ou0 = small.tile([P, QW], FP32, tag="ou0")
        ou1 = small.tile([P, QW], FP32, tag="ou1")
        nc.vector.tensor_copy(ou0[0:DK + 32, :], psOs[0][0:DK + 32, :])
        nc.vector.tensor_copy(ou1[:], psOs[1][:])
        # on-chip 1/rowsum broadcast, zero DMAs: each head's rowsum row (64
        # for head 2j, 0 for 2j+1) already sits inside a legal 32-row block,
        # so DVE 32x32-block transposes lift it onto 32 partitions (columns
        # =0 mod 32), reciprocal runs on just those columns (fp32 -> fp16,
        # ~5e-4 rel, well inside budget), a transpose back restores the row,
        # and two K=1 fp16 matmuls broadcast it to the 64 psum rows of each
        # head's O block.  Junk rows/cols flow through unread.
        tc_t = nrm.tile([P, QW], FP32, tag="tc")
        nc.vector.transpose(tc_t[DK:DK + 32, :], ou0[DK:DK + 32, :])
        nc.vector.transpose(tc_t[0:32, :], ou1[0:32, :])
        tc2 = tc2P[:, unit_no[0] % 2, :]
        unit_no[0] += 1
        with nc.allow_low_precision(reason="1/rowsum in fp16"):
            nc.vector.reciprocal(
                tc2[DK:DK + 32].rearrange("p (b t) -> p b t", t=32)[:, :, 0:1],
                tc_t[DK:DK + 32].rearrange("p (b t) -> p b t", t=32)[:, :, 0:1])
            nc.vector.reciprocal(
                tc2[0:32].rearrange("p (b t) -> p b t", t=32)[:, :, 0:1],
                tc_t[0:32].rearrange("p (b t) -> p b t", t=32)[:, :, 0:1])
        trec = nrm.tile([P, QW], F16, tag="trec")
        nc.vector.transpose(trec[DK:DK + 32, :], tc2[DK:DK + 32, :])
        nc.vector.transpose(trec[0:32, :], tc2[0:32, :])
        bc_ps = psB.tile([P, QW], FP32, tag="b", name="bc")
        nc.tensor.matmul(bc_ps[0:DK, :], lhsT=E_sb[DK:DK + 1, :],
                         rhs=trec[DK:DK + 1, :], start=True, stop=True)
        nc.tensor.matmul(bc_ps[DK:P, :], lhsT=E_sb[0:1, :],
                         rhs=trec[0:1, :], start=True, stop=True)
        nc.vector.tensor_mul(ot[0:DK, j, qcols], ou0[0:DK, :], bc_ps[0:DK, :])
        nc.vector.tensor_mul(ot[DK:P, j, qcols], ou1[DK:P, :], bc_ps[DK:P, :])

    def final_qb(qb):
        for oc in range(D // P):
            ps = psB.tile([P, QW], FP32, tag="b")
            for dc in range(DC):
                nc.tensor.matmul(
                    ps[:],
                    lhsT=w_o[:, dc, oc * P:(oc + 1) * P],
                    rhs=ot[:, dc, qb * QW:(qb + 1) * QW],
                    start=(dc == 0), stop=(dc == DC - 1))
            y_sb = ypool.tile([P, QW], FP32, tag="y")
            # explicit DVE: finals now run alongside exps, and nc.any would
            # put these copies on the exp-critical ACT engine
            nc.vector.tensor_copy(y_sb[:], ps[:])
            nc.sync.dma_start(
                yt[oc * P:(oc + 1) * P, qb * QW:(qb + 1) * QW], y_sb[:])

    y012 = opool.tile([P, D // P, QW], FP32)

    def final_qb_partial(qb):
        """d-chunks 0..2 of the last q-block's output projection, runnable
        before the last attention unit: shrinks the tail to one matmul."""
        for oc in range(D // P):
            ps = psB.tile([P, QW], FP32, tag="b")
            for dc in range(DC - 1):
                nc.tensor.matmul(
                    ps[:],
                    lhsT=w_o[:, dc, oc * P:(oc + 1) * P],
                    rhs=ot[:, dc, qb * QW:(qb + 1) * QW],
                    start=(dc == 0), stop=(dc == DC - 2))
            nc.vector.tensor_copy(y012[:, oc, :], ps[:])

    def final_qb_tail(qb):
        for oc in range(D // P):
            ps = psB.tile([P, QW], FP32, tag="b")
            nc.tensor.matmul(
                ps[:],
                lhsT=w_o[:, DC - 1, oc * P:(oc + 1) * P],
                rhs=ot[:, DC - 1, qb * QW:(qb + 1) * QW],
                start=True, stop=True)
            y_sb = ypool.tile([P, QW], FP32, tag="y")
            nc.vector.tensor_add(y_sb[:], ps[:], y012[:, oc, :])
            nc.sync.dma_start(
                yt[oc * P:(oc + 1) * P, qb * QW:(qb + 1) * QW], y_sb[:])

    def compute_once():
        # Emission order == scheduler priority.  Software-pipelined: the
        # scores+exp of unit u+1 outrank the PV of unit u, so the ACT exp
        # stream (the co-bottleneck) never waits for PE catch-up work;
        # PV/projections/finals fill PE slack between paced scores tiles.
        # j-major spreads the Q/K projection chunks across the kernel (each
        # chunk c is consumed by 4 consecutive units) instead of packing all
        # four into q-block 0's window; finals land inside head-pair group 3,
        # one per unit, with qb=3's final split so the tail after the last
        # exp is just one accumulation step.
        units = [(qb, j) for j in range(DC) for qb in range(QB)]
        for tb in range(QB):
            qk_proj_tb(0, tb)
            scores_exp(0, 0, range(4 * tb, 4 * tb + 4))
        scores_exp(1, 0)
        v_proj()
        scores_exp(2, 0)
        qk_proj_chunk(1)
        for i, (qb, j) in enumerate(units):
            if i + 3 < len(units):
                scores_exp(*units[i + 3])
            if i == 4:
                qk_proj_chunk(2)
            elif i == 8:
                qk_proj_chunk(3)
            elif i == 12 and ABLATE["final"]:
                final_qb_partial(3)
            pv_norm(qb, j)
            if j == DC - 1 and ABLATE["final"]:
                if qb == QB - 1:
                    final_qb_tail(qb)
                else:
                    final_qb(qb)

    for _ in range(repeat):
        compute_once()


def build_module(repeat=1):
    nc = bacc.Bacc("TRN2", target_bir_lowering=False, debug=False)
    xt = nc.dram_tensor("xt", [D, S], BF, kind="ExternalInput").ap()
    wq = nc.dram_tensor("wq", [D, DS], BF, kind="ExternalInput").ap()
    wk = nc.dram_tensor("wk", [D, DS], BF, kind="ExternalInput").ap()
    wv = nc.dram_tensor("wv", [D, DS], BF, kind="ExternalInput").ap()
    wo = nc.dram_tensor("wo", [DS, D], BF, kind="ExternalInput").ap()
    bq = nc.dram_tensor("bq", [DS], FP32, kind="ExternalInput").ap()
    bk = nc.dram_tensor("bk", [DS], FP32, kind="ExternalInput").ap()
    yt = nc.dram_tensor("yt", [D, S], FP32, kind="ExternalOutput").ap()
    with tile.TileContext(nc) as tc:
        with ExitStack() as ctx:
            build_tile_kernel(ctx, tc, xt, wq, wk, wv, wo, bq, bk, yt,
                              repeat=repeat)
    nc.compile()
    return nc


def _collect_io(nc):
    partition_name = (nc.partition_id_tensor.name
                      if nc.partition_id_tensor else None)
    in_names, out_names, out_avals = [], [], []
    for alloc in nc.m.functions[0].allocations:
        if not isinstance(alloc, mybir.MemoryLocationSet):
            continue
        name = alloc.memorylocations[0].name
        if alloc.kind == "ExternalInput":
            if name != partition_name:
                in_names.append(name)
        elif alloc.kind == "ExternalOutput":
            out_names.append(name)
            out_avals.append(jax.core.ShapedArray(
                tuple(alloc.tensor_shape), mybir.dt.np(alloc.dtype)))
    return in_names, out_names, out_avals, partition_name


def make_runner(nc, donate=False):
    """Multi-core PJRT runner (the run_bass_via_pjrt path, but with the
    jitted executable retained so repeated calls don't re-lower).

    donate=False: the kernel writes every element of its outputs, so the
    zero output-operands never need to be donated; keeping them allows the
    same device-resident args to be re-used for repeated timed calls."""
    bass2jax.install_neuronx_cc_hook()
    in_names, out_names, out_avals, partition_name = _collect_io(nc)
    n_params, n_outs = len(in_names), len(out_names)
    all_names = in_names + out_names
    if partition_name is not None:
        all_names = all_names + [partition_name]

    def _body(*args):
        operands = list(args)
        if partition_name is not None:
            operands.append(bass2jax.partition_id_tensor())
        outs = bass2jax._bass_exec_p.bind(
            *operands,
            out_avals=tuple(out_avals),
            in_names=tuple(all_names),
            out_names=tuple(out_names),
            lowering_input_output_aliases=(),
            sim_require_finite=True,
            sim_require_nnan=True,
            nc=nc,
        )
        return tuple(outs)

    devices = jax.devices()[:N_CORES]
    mesh = Mesh(np.asarray(devices), ("core",))
    jit_kwargs = dict(keep_unused=True)
    if donate:
        jit_kwargs["donate_argnums"] = tuple(range(n_params, n_params + n_outs))
    sharded = jax.jit(
        shard_map(_body, mesh=mesh,
                  in_specs=(PartitionSpec("core"),) * (n_params + n_outs),
                  out_specs=(PartitionSpec("core"),) * n_outs,
                  check_rep=False),
        **jit_kwargs)

    def host_args(in_maps):
        concat_in = [
            np.concatenate([np.asarray(m[name]) for m in in_maps], axis=0)
            for name in in_names]
        concat_zeros = [
            np.zeros((N_CORES * a.shape[0],) + tuple(a.shape[1:]), a.dtype)
            for a in out_avals]
        return concat_in + concat_zeros

    def device_args(in_maps):
        from jax.sharding import NamedSharding
        args = host_args(in_maps)
        return [
            jax.device_put(a, NamedSharding(
                mesh, PartitionSpec("core", *(None,) * (a.ndim - 1))))
            for a in args]

    def run(in_maps, args=None):
        if args is None:
            args = host_args(in_maps)
        out_arrs = sharded(*args)
        return [
            {name: np.asarray(out_arrs[i]).reshape(
                (N_CORES,) + tuple(out_avals[i].shape))[c]
             for i, name in enumerate(out_names)}
            for c in range(N_CORES)]

    run.in_names = in_names
    run.out_names = out_names
    run.out_avals = out_avals
    run.sharded = sharded
    run.mesh = mesh
    run.host_args = host_args
    run.device_args = device_args
    return run


def shard_inputs(inputs):
    """Full problem inputs -> 8 per-core input maps (host-side prep)."""
    x = np.asarray(inputs["x"], dtype=np.float32)
    Wq = np.asarray(inputs["Wq"], dtype=np.float32)
    Wk = np.asarray(inputs["Wk"], dtype=np.float32)
    Wv = np.asarray(inputs["Wv"], dtype=np.float32)
    Wo = np.asarray(inputs["Wo"], dtype=np.float32)
    bq = np.asarray(inputs["bq"], dtype=np.float32)
    bk = np.asarray(inputs["bk"], dtype=np.float32)
    in_maps = []
    for b in range(B):
        xt_b = np.ascontiguousarray(x[b].T).astype(BF16)
        for hg in range(2):
            sl = slice(hg * DS, (hg + 1) * DS)
            in_maps.append({
                "xt": xt_b,
                "wq": np.ascontiguousarray(Wq[:, sl]).astype(BF16),
                "wk": np.ascontiguousarray(Wk[:, sl]).astype(BF16),
                "wv": np.ascontiguousarray(Wv[:, sl]).astype(BF16),
                "wo": np.ascontiguousarray(Wo[sl, :]).astype(BF16),
                "bq": np.ascontiguousarray(bq[sl]),
                "bk": np.ascontiguousarray(bk[sl]),
            })
    return in_maps


def gather_output(results, inputs):
    Wo = np.asarray(inputs["Wo"], dtype=np.float32)
    bv = np.asarray(inputs["bv"], dtype=np.float32)
    bo = np.asarray(inputs["bo"], dtype=np.float32)
    bias = bo + bv @ Wo  # V-bias passes through softmax (rows of P sum to 1)
    out = np.empty((B, S, D), dtype=np.float32)
    for b in range(B):
        acc = results[2 * b]["yt"] + results[2 * b + 1]["yt"]  # [D, S]
        out[b] = acc.T + bias
    return out


_CACHE = {}


def _get_runner():
    if "runner" not in _CACHE:
        nc = build_module()
        _CACHE["nc"] = nc
        _CACHE["runner"] = make_runner(nc)
    return _CACHE["runner"]


def kernel(**inputs) -> np.ndarray:
    runner = _get_runner()
    in_maps = shard_inputs(inputs)
    results = runner(in_maps)
    return gather_output(results, inputs)

